# revision 6
# baseline (speedup 1.0000x reference)
"""Causal multi-head attention (RoPE) forward for Trainium2, sharded over 8 NeuronCores.

Problem (hardcoded): B=2, S=2048, E=128, H=16, D=128, inner=2048.
  out = softmax(causal(rope(q@Wq) @ rope(q@Wk).T / sqrt(D))) @ (q@Wv) @ Wo

Sharding: tensor-parallel over heads - core c owns heads {2c, 2c+1} for both
batches (4 attention units/core). Host combines per-head partial outputs.

Design notes (v3):
 - W_o folded into V on the host: Wf_h = W_v[:,h] @ W_o[h,:] ([E,E] per head).
   The transposed AV accumulation avT[E,q] = sum_c vwo_c.T @ probsT_c IS the
   head's output partial: no on-device W_o matmul, transposes, or head sum.
 - Softmax normalization on the HOST: device ships unnormalized avT (f16) and
   denominator partials dens[t,q] (sum of exp tiles over chunks); host
   divides and sums heads/cores.
 - All matmuls f16. Scores for t-chunk pairs -> [128,1024] 2-bank PSUM tiles,
   one full-width exp per pair (ACT). Diagonal-window chunks: partial-width
   score matmul [jlo:512]; exp of the junk region reads stale PSUM (finite,
   old scores) and is zeroed by gpsimd memsets before the den tree.
 - Causal tril mask ([128,128] per diag chunk) on gpsimd; rope fold-adds on
   gpsimd; everything PSUM-touching on DVE (rope pair-muls, vwo/fin evicts);
   den tree (wide f16 adds, 2x_1P mode) on DVE.
 - PSUM: score/stage-B shared pool 3x[128,1024] (6 banks) + 2 avT banks.
 - stage_b(b=1) interleaved into stage_c(b=0) at pair granularity to fill PE
   gaps; AV matmuls lag scores by 2 pairs so PE never waits on ACT.
"""

import os
import sys
import numpy as np

for _p in ("/root/.axon_site", "/root/.axon_site/_ro/trn_rl_repo",
           "/root/.axon_site/_ro/pypackages", "/opt/trn_rl_repo"):
    if os.path.isdir(_p) and _p not in sys.path:
        sys.path.append(_p)

from contextlib import ExitStack

import concourse.bacc as bacc
import concourse.mybir as mybir
import concourse.tile as tile
from concourse import bass_utils

F32 = mybir.dt.float32
F16 = mybir.dt.float16
AF = mybir.ActivationFunctionType

B, S, E = 2, 2048, 128
H, D = 16, 128
NCORES = 8
HPC = H // NCORES          # heads per core = 2
WIN = 512                  # q-window
NW = S // WIN              # windows per batch = 4
SCALE = 1.0 / np.sqrt(D)

_CACHE = {}


def _build():
    nc = bacc.Bacc("TRN2", target_bir_lowering=False, debug=False)

    qT_d = nc.dram_tensor("qT", [E, B * S], F16, kind="ExternalInput").ap()
    wqk_d = nc.dram_tensor("wqk", [E, 8 * D], F16, kind="ExternalInput").ap()
    wf2_d = nc.dram_tensor("wf2", [E, HPC * E], F16, kind="ExternalInput").ap()
    cs_d = nc.dram_tensor("cs", [D, 2 * S], F16, kind="ExternalInput").ap()
    tril_d = nc.dram_tensor("trilT", [128, 128], F16, kind="ExternalInput").ap()
    fins_d = nc.dram_tensor("fins", [HPC * E, B * S], F16, kind="ExternalOutput").ap()
    dens_d = nc.dram_tensor("dens", [128, B * NW * HPC * WIN], F16,
                            kind="ExternalOutput").ap()

    with tile.TileContext(nc) as tc, ExitStack() as ctx:
        const = ctx.enter_context(tc.tile_pool(name="const", bufs=1))
        qkp = ctx.enter_context(tc.tile_pool(name="qkp", bufs=1))
        vhp = ctx.enter_context(tc.tile_pool(name="vhp", bufs=1))
        t12p = ctx.enter_context(tc.tile_pool(name="t12p", bufs=3))
        expp = ctx.enter_context(tc.tile_pool(name="expp", bufs=20))
        finp = ctx.enter_context(tc.tile_pool(name="finp", bufs=4))
        ps_s = ctx.enter_context(tc.tile_pool(name="ps_s", bufs=3, space="PSUM"))
        ps_av = ctx.enter_context(tc.tile_pool(name="ps_av", bufs=2, space="PSUM"))

        # ---- constant loads ----
        qt_w = []
        for i in range(B * NW):
            t = const.tile([128, WIN], F16, tag=f"qt{i}")
            nc.sync.dma_start(t[:], qT_d[:, i * WIN:(i + 1) * WIN])
            qt_w.append(t)
        wqk_t = const.tile([128, 8 * D], F16, tag="wqk")
        nc.sync.dma_start(wqk_t[:], wqk_d[:])
        wf2_t = const.tile([128, HPC * E], F16, tag="wf2")
        nc.sync.dma_start(wf2_t[:], wf2_d[:])
        cs_t = const.tile([128, 2 * S], F16, tag="cs")
        nc.sync.dma_start(cs_t[:], cs_d[:])
        tril_t = const.tile([128, 128], F16, tag="trilT")
        nc.sync.dma_start(tril_t[:], tril_d[:])

        # persistent rope'd q/k: (u, kind, w) -> [128, WIN] f16 (feature-major)
        qk = {}
        for u in range(B * HPC):
            for kind in range(2):
                for w in range(NW):
                    qk[(u, kind, w)] = qkp.tile(
                        [128, WIN], F16, tag=f"qk{u}_{kind}_{w}",
                        name=f"qk{u}_{kind}_{w}")
        # persistent vwo (V@Wo fused), token-major: vh[b][:, c*256 + hl*128]
        vh = {}
        for b in range(B):
            vh[b] = vhp.tile([128, 4096], F16, tag=f"vh{b}", name=f"vh{b}")

        def stage_b_quanta(b):
            """Yield stage-B emission quanta for batch b (callables)."""
            for w in range(NW):
                i = b * NW + w
                for hl in range(HPC):
                    for kind in range(2):
                        yield lambda w=w, i=i, hl=hl, kind=kind: _rope_unit(
                            b, w, i, hl, kind)
                yield lambda w=w, i=i: _v_unit(b, w, i)

        def _rope_unit(b, w, i, hl, kind):
            u = b * HPC + hl
            ja = (hl * 2 + kind) * 256
            ps = ps_s.tile([128, 1024], F32, tag="ps_s",
                           name=f"psb{b}_{w}_{hl}_{kind}")
            nc.tensor.matmul(ps[:, 0:512], wqk_t[:, ja:ja + 128], qt_w[i][:])
            nc.tensor.matmul(ps[:, 512:1024],
                             wqk_t[:, ja + 128:ja + 256], qt_w[i][:])
            t12 = t12p.tile([128, 1024], F16, tag="t12",
                            name=f"t12_{b}_{w}_{hl}_{kind}")
            nc.vector.tensor_mul(
                t12[:], ps[:], cs_t[:, w * 1024:(w + 1) * 1024])
            nc.gpsimd.tensor_add(
                qk[(u, kind, w)][:], t12[:, 0:512], t12[:, 512:1024])

        def _v_unit(b, w, i):
            psv = ps_s.tile([128, 1024], F32, tag="ps_s", name=f"psv{b}_{w}")
            for sub in range(4):
                nc.tensor.matmul(psv[:, sub * 256:(sub + 1) * 256],
                                 qt_w[i][:, sub * 128:(sub + 1) * 128],
                                 wf2_t[:])
            nc.vector.tensor_copy(vh[b][:, w * 1024:(w + 1) * 1024], psv[:])

        def stage_c(b, W, filler=None):
            npair = 2 * W + 2
            avs = {}
            for hl in range(HPC):
                avs[hl] = ps_av.tile([128, WIN], F32, tag="av",
                                     name=f"av{b}_{W}_{hl}")
            e2s = {hl: [] for hl in range(HPC)}
            pend_av = []
            step = 0
            for p in range(npair):
                for hl in range(HPC):
                    u = b * HPC + hl
                    ps = ps_s.tile([128, 1024], F32, tag="ps_s",
                                   name=f"ps_{b}_{W}_{hl}_{p}")
                    for h2 in range(2):
                        c = 2 * p + h2
                        kw, ks = c // 4, c % 4
                        jlo = max(0, 128 * c - 512 * W)
                        nc.tensor.matmul(
                            ps[:, h2 * 512 + jlo:(h2 + 1) * 512],
                            qk[(u, 1, kw)][:, ks * 128:(ks + 1) * 128],
                            qk[(u, 0, W)][:, jlo:WIN])
                    e2 = expp.tile([128, 1024], F16, tag="e2",
                                   name=f"e_{b}_{W}_{hl}_{p}")
                    if p < npair - 2:
                        nc.scalar.activation(e2[:], ps[:], AF.Exp,
                                             scale=float(SCALE))
                    else:
                        # diagonal pair: ranged exp + tril mask + zero junk
                        for h2 in range(2):
                            c = 2 * p + h2
                            jlo = 128 * (c - 4 * W)
                            lo, hi = h2 * 512 + jlo, (h2 + 1) * 512
                            nc.scalar.activation(e2[:, lo:hi], ps[:, lo:hi],
                                                 AF.Exp, scale=float(SCALE))
                            nc.gpsimd.tensor_mul(
                                e2[:, lo:lo + 128], e2[:, lo:lo + 128],
                                tril_t[:])
                            if jlo > 0:
                                nc.gpsimd.memset(
                                    e2[:, h2 * 512:lo], 0.0)
                    e2s[hl].append(e2)
                    pend_av.append((hl, p, e2))
                    if len(pend_av) > 4:
                        _emit_av(b, W, pend_av.pop(0), avs)
                    step += 1
                    if filler is not None and step % 2 == 0:
                        q = next(filler, None)
                        if q is not None:
                            q()
            while pend_av:
                _emit_av(b, W, pend_av.pop(0), avs)

            for hl in range(HPC):
                # den tree: wide in-place f16 adds over pair tiles
                cur = list(e2s[hl])
                while len(cur) > 1:
                    nxt = []
                    for j in range(0, len(cur) - 1, 2):
                        nc.vector.tensor_add(cur[j][:], cur[j][:],
                                             cur[j + 1][:])
                        nxt.append(cur[j])
                    if len(cur) % 2:
                        nxt.append(cur[-1])
                    cur = nxt
                nc.vector.tensor_add(cur[0][:, 0:512], cur[0][:, 0:512],
                                     cur[0][:, 512:1024])
                blk = ((b * NW + W) * HPC + hl) * WIN
                nc.sync.dma_start(dens_d[:, blk:blk + WIN], cur[0][:, 0:512])
                fin = finp.tile([128, WIN], F16, tag="fin",
                                name=f"fin{b}_{W}_{hl}")
                nc.vector.tensor_copy(fin[:], avs[hl][:])
                nc.sync.dma_start(
                    fins_d[hl * E:(hl + 1) * E,
                           b * S + W * WIN:b * S + (W + 1) * WIN], fin[:])

        def _emit_av(b, W, item, avs):
            hl, p, e2 = item
            for h2 in range(2):
                c = 2 * p + h2
                jlo = max(0, 128 * c - 512 * W)
                nc.tensor.matmul(
                    avs[hl][:, jlo:WIN],
                    vh[b][:, c * 256 + hl * 128:c * 256 + (hl + 1) * 128],
                    e2[:, h2 * 512 + jlo:(h2 + 1) * 512],
                    start=(c == 0), stop=(c == 4 * W + 3))

        for w in range(NW):
            for q in [None]:
                pass
            i = 0 * NW + w
            for hl in range(HPC):
                for kind in range(2):
                    _rope_unit(0, w, i, hl, kind)
            _v_unit(0, w, i)
        filler = stage_b_quanta(1)
        for W in range(NW):
            stage_c(0, W, filler=filler)
        for q in filler:
            q()
        for W in range(NW):
            stage_c(1, W)

    nc.compile()
    return nc


def _get_nc():
    if "nc" not in _CACHE:
        _CACHE["nc"] = _build()
    return _CACHE["nc"]


def _host_inputs(q, W_q, W_k, W_v, W_o):
    """Shared (core-independent) host-side prep."""
    qT = np.ascontiguousarray(q.reshape(B * S, E).T).astype(np.float16)

    half = D // 2
    inv = (1.0 / (10000.0 ** (np.arange(half, dtype=np.float64) * 2.0 / D)))
    ang = np.arange(S, dtype=np.float64)[None, :] * inv[:, None]   # [half, S]
    cosT = np.repeat(np.cos(ang), 2, axis=0)                       # [D, S]
    sinT = np.repeat(np.sin(ang), 2, axis=0)
    cs = np.empty((D, 2 * S), dtype=np.float16)
    for w in range(NW):
        cs[:, w * 1024:w * 1024 + 512] = cosT[:, w * 512:(w + 1) * 512]
        cs[:, w * 1024 + 512:(w + 1) * 1024] = sinT[:, w * 512:(w + 1) * 512]
    # tril[t, j] = 1 if j >= t  (keep q >= t within the diagonal block)
    tril = np.tril(np.ones((128, 128), dtype=np.float16)).T
    tril = np.ascontiguousarray(tril)
    return qT, cs, tril


def _swap_neg(w):
    """W' columns: w2[:, 2i] = -w[:, 2i+1], w2[:, 2i+1] = w[:, 2i]."""
    w2 = np.empty_like(w)
    w2[:, 0::2] = -w[:, 1::2]
    w2[:, 1::2] = w[:, 0::2]
    return w2


def kernel(q, W_q, W_k, W_v, W_o):
    q = np.asarray(q, dtype=np.float32)
    W_q = np.asarray(W_q, dtype=np.float64)
    W_k = np.asarray(W_k, dtype=np.float64)
    W_v = np.asarray(W_v, dtype=np.float64)
    W_o = np.asarray(W_o, dtype=np.float64)

    nc = _get_nc()
    qT, cs, tril = _host_inputs(q, W_q, W_k, W_v, W_o)

    in_maps = []
    for c in range(NCORES):
        wqk = np.empty((E, 8 * D), dtype=np.float16)
        wf2 = np.empty((E, HPC * E), dtype=np.float16)
        for hl in range(HPC):
            h = c * HPC + hl
            for kind, Wm in ((0, W_q), (1, W_k)):
                wslc = Wm[:, h * D:(h + 1) * D]
                ja = (hl * 2 + kind) * 256
                wqk[:, ja:ja + D] = wslc
                wqk[:, ja + D:ja + 2 * D] = _swap_neg(wslc)
            wf2[:, hl * E:(hl + 1) * E] = (
                W_v[:, h * D:(h + 1) * D] @ W_o[h * D:(h + 1) * D, :])
        in_maps.append({
            "qT": qT, "wqk": wqk, "wf2": wf2, "cs": cs, "trilT": tril,
        })

    res = bass_utils.run_bass_kernel_spmd(
        nc, in_maps, core_ids=list(range(NCORES)),
        trace=bool(int(os.environ.get("KERNEL_TRACE", "0"))))
    _CACHE["last_result"] = res

    out = np.zeros((B, S, E), dtype=np.float64)
    for r in res.results:
        out += _combine(r)
    return out.astype(np.float32)


def _combine(r):
    """Host-side normalization + head sum for one core's outputs."""
    fins = r["fins"].astype(np.float64).reshape(HPC, E, B, S)   # [hl,e,b,q]
    dens = r["dens"].astype(np.float64).reshape(
        128, B, NW, HPC, WIN)                                   # [t,b,W,hl,j]
    den = dens.sum(axis=0)                                      # [b,W,hl,j]
    den = den.transpose(0, 2, 1, 3).reshape(B, HPC, S)          # [b,hl,q]
    return np.einsum("lebq->bqe", fins / den.transpose(1, 0, 2)[:, None, :, :])


# revision 11
# speedup vs baseline: 1.2230x; 1.2230x over previous
"""Causal multi-head attention (RoPE) forward for Trainium2, sharded over 8 NeuronCores.

Problem (hardcoded): B=2, S=2048, E=128, H=16, D=128, inner=2048.
  out = softmax(causal(rope(q@Wq) @ rope(q@Wk).T / sqrt(D))) @ (q@Wv) @ Wo

Sharding: tensor-parallel over heads - core c owns heads {2c, 2c+1} for both
batches (4 attention units/core). Host combines per-head partial outputs.

Design notes (v3):
 - W_o folded into V on the host: Wf_h = W_v[:,h] @ W_o[h,:] ([E,E] per head).
   The transposed AV accumulation avT[E,q] = sum_c vwo_c.T @ probsT_c IS the
   head's output partial: no on-device W_o matmul, transposes, or head sum.
 - Softmax normalization on the HOST: device ships unnormalized avT (f16) and
   denominator partials dens[t,q] (sum of exp tiles over chunks); host
   divides and sums heads/cores.
 - All matmuls f16. Scores for t-chunk pairs -> [128,1024] 2-bank PSUM tiles,
   one full-width exp per pair (ACT). Diagonal-window chunks: partial-width
   score matmul [jlo:512]; exp of the junk region reads stale PSUM (finite,
   old scores) and is zeroed by gpsimd memsets before the den tree.
 - Causal tril mask ([128,128] per diag chunk) on gpsimd; rope fold-adds on
   gpsimd; everything PSUM-touching on DVE (rope pair-muls, vwo/fin evicts);
   den tree (wide f16 adds, 2x_1P mode) on DVE.
 - PSUM: score/stage-B shared pool 3x[128,1024] (6 banks) + 2 avT banks.
 - stage_b(b=1) interleaved into stage_c(b=0) at pair granularity to fill PE
   gaps; AV matmuls lag scores by 2 pairs so PE never waits on ACT.
"""

import os
import sys
import numpy as np

for _p in ("/root/.axon_site", "/root/.axon_site/_ro/trn_rl_repo",
           "/root/.axon_site/_ro/pypackages", "/opt/trn_rl_repo"):
    if os.path.isdir(_p) and _p not in sys.path:
        sys.path.append(_p)

from contextlib import ExitStack

import concourse.bacc as bacc
import concourse.mybir as mybir
import concourse.tile as tile
from concourse import bass_utils

F32 = mybir.dt.float32
F16 = mybir.dt.float16
AF = mybir.ActivationFunctionType

B, S, E = 2, 2048, 128
H, D = 16, 128
NCORES = 8
HPC = H // NCORES          # heads per core = 2
WIN = 512                  # q-window
NW = S // WIN              # windows per batch = 4
SCALE = 1.0 / np.sqrt(D)

_CACHE = {}


def _build():
    nc = bacc.Bacc("TRN2", target_bir_lowering=False, debug=False)

    qT_d = nc.dram_tensor("qT", [E, B * S], F16, kind="ExternalInput").ap()
    wqk_d = nc.dram_tensor("wqk", [E, 8 * D], F16, kind="ExternalInput").ap()
    wf2_d = nc.dram_tensor("wf2", [E, HPC * E], F16, kind="ExternalInput").ap()
    cs_d = nc.dram_tensor("cs", [D, 2 * S], F16, kind="ExternalInput").ap()
    tril_d = nc.dram_tensor("trilT", [128, 128], F16, kind="ExternalInput").ap()
    fins_d = nc.dram_tensor("fins", [HPC * E, B * S], F16, kind="ExternalOutput").ap()
    dens_d = nc.dram_tensor("dens", [128, B * NW * HPC * WIN], F16,
                            kind="ExternalOutput").ap()

    with tile.TileContext(nc) as tc, ExitStack() as ctx:
        const = ctx.enter_context(tc.tile_pool(name="const", bufs=1))
        qkp = ctx.enter_context(tc.tile_pool(name="qkp", bufs=1))
        vhp = ctx.enter_context(tc.tile_pool(name="vhp", bufs=1))
        t12p = ctx.enter_context(tc.tile_pool(name="t12p", bufs=3))
        expp = ctx.enter_context(tc.tile_pool(name="expp", bufs=20))
        finp = ctx.enter_context(tc.tile_pool(name="finp", bufs=4))
        ps_s = ctx.enter_context(tc.tile_pool(name="ps_s", bufs=3, space="PSUM"))
        ps_av = ctx.enter_context(tc.tile_pool(name="ps_av", bufs=2, space="PSUM"))

        # ---- constant loads (ordered so window-0 work can start ASAP) ----
        wqk_t = const.tile([128, 8 * D], F16, tag="wqk")
        nc.sync.dma_start(wqk_t[:], wqk_d[:])
        qt_w = []
        for i in range(B * NW):
            t = const.tile([128, WIN], F16, tag=f"qt{i}")
            qt_w.append(t)
        nc.sync.dma_start(qt_w[0][:], qT_d[:, 0:WIN])
        wf2_t = const.tile([128, HPC * E], F16, tag="wf2")
        nc.sync.dma_start(wf2_t[:], wf2_d[:])
        cs_t = const.tile([128, 2 * S], F16, tag="cs")
        nc.scalar.dma_start(cs_t[:, 0:1024], cs_d[:, 0:1024])
        tril_t = const.tile([128, 128], F16, tag="trilT")
        nc.sync.dma_start(tril_t[:], tril_d[:])
        for i in range(1, B * NW):
            nc.sync.dma_start(qt_w[i][:], qT_d[:, i * WIN:(i + 1) * WIN])
        nc.scalar.dma_start(cs_t[:, 1024:2 * S], cs_d[:, 1024:2 * S])

        # persistent rope'd q/k: (u, kind, w) -> [128, WIN] f16 (feature-major)
        qk = {}
        for u in range(B * HPC):
            for kind in range(2):
                for w in range(NW):
                    qk[(u, kind, w)] = qkp.tile(
                        [128, WIN], F16, tag=f"qk{u}_{kind}_{w}",
                        name=f"qk{u}_{kind}_{w}")
        # persistent vwo (V@Wo fused), token-major: vh[b][:, c*256 + hl*128]
        vh = {}
        for b in range(B):
            vh[b] = vhp.tile([128, 4096], F16, tag=f"vh{b}", name=f"vh{b}")

        def stage_b_quanta(units):
            """Yield stage-B emission quanta for (b, w) units (callables)."""
            for b, w in units:
                i = b * NW + w
                for hl in range(HPC):
                    for kind in range(2):
                        yield lambda b=b, w=w, i=i, hl=hl, kind=kind: \
                            _rope_unit(b, w, i, hl, kind)
                yield lambda b=b, w=w, i=i: _v_unit(b, w, i)

        def _rope_unit(b, w, i, hl, kind):
            u = b * HPC + hl
            ja = (hl * 2 + kind) * 256
            ps = ps_s.tile([128, 1024], F32, tag="ps_s",
                           name=f"psb{b}_{w}_{hl}_{kind}")
            nc.tensor.matmul(ps[:, 0:512], wqk_t[:, ja:ja + 128], qt_w[i][:])
            nc.tensor.matmul(ps[:, 512:1024],
                             wqk_t[:, ja + 128:ja + 256], qt_w[i][:])
            t12 = t12p.tile([128, 1024], F16, tag="t12",
                            name=f"t12_{b}_{w}_{hl}_{kind}")
            nc.vector.tensor_mul(
                t12[:], ps[:], cs_t[:, w * 1024:(w + 1) * 1024])
            nc.gpsimd.tensor_add(
                qk[(u, kind, w)][:], t12[:, 0:512], t12[:, 512:1024])

        def _v_unit(b, w, i):
            psv = ps_s.tile([128, 1024], F32, tag="ps_s", name=f"psv{b}_{w}")
            for sub in range(4):
                nc.tensor.matmul(psv[:, sub * 256:(sub + 1) * 256],
                                 qt_w[i][:, sub * 128:(sub + 1) * 128],
                                 wf2_t[:])
            nc.vector.tensor_copy(vh[b][:, w * 1024:(w + 1) * 1024], psv[:])

        def stage_c(b, W, filler=None):
            npair = 2 * W + 2
            avs = {}
            for hl in range(HPC):
                avs[hl] = ps_av.tile([128, WIN], F32, tag="av",
                                     name=f"av{b}_{W}_{hl}")
            e2s = {hl: [] for hl in range(HPC)}
            pend_av = []
            step = 0
            for p in range(npair):
                for hl in range(HPC):
                    u = b * HPC + hl
                    ps = ps_s.tile([128, 1024], F32, tag="ps_s",
                                   name=f"ps_{b}_{W}_{hl}_{p}")
                    for h2 in range(2):
                        c = 2 * p + h2
                        kw, ks = c // 4, c % 4
                        jlo = max(0, 128 * c - 512 * W)
                        nc.tensor.matmul(
                            ps[:, h2 * 512 + jlo:(h2 + 1) * 512],
                            qk[(u, 1, kw)][:, ks * 128:(ks + 1) * 128],
                            qk[(u, 0, W)][:, jlo:WIN])
                    e2 = expp.tile([128, 1024], F16, tag="e2",
                                   name=f"e_{b}_{W}_{hl}_{p}")
                    if p < npair - 2:
                        nc.scalar.activation(e2[:], ps[:], AF.Exp,
                                             scale=float(SCALE))
                    else:
                        # diagonal pair: ranged exp + tril mask + zero junk
                        for h2 in range(2):
                            c = 2 * p + h2
                            jlo = 128 * (c - 4 * W)
                            lo, hi = h2 * 512 + jlo, (h2 + 1) * 512
                            nc.scalar.activation(e2[:, lo:hi], ps[:, lo:hi],
                                                 AF.Exp, scale=float(SCALE))
                            nc.gpsimd.tensor_mul(
                                e2[:, lo:lo + 128], e2[:, lo:lo + 128],
                                tril_t[:])
                            if jlo > 0:
                                nc.gpsimd.memset(
                                    e2[:, h2 * 512:lo], 0.0)
                    e2s[hl].append(e2)
                    pend_av.append((hl, p, e2))
                    if len(pend_av) > 4:
                        _emit_av(b, W, pend_av.pop(0), avs)
                    step += 1
                    if filler is not None:
                        q = next(filler, None)
                        if q is not None:
                            q()
            while pend_av:
                _emit_av(b, W, pend_av.pop(0), avs)

            for hl in range(HPC):
                # den tree: wide in-place f16 adds over pair tiles
                cur = list(e2s[hl])
                while len(cur) > 1:
                    nxt = []
                    for j in range(0, len(cur) - 1, 2):
                        nc.vector.tensor_add(cur[j][:], cur[j][:],
                                             cur[j + 1][:])
                        nxt.append(cur[j])
                    if len(cur) % 2:
                        nxt.append(cur[-1])
                    cur = nxt
                nc.gpsimd.tensor_add(cur[0][:, 0:512], cur[0][:, 0:512],
                                     cur[0][:, 512:1024])
                blk = ((b * NW + W) * HPC + hl) * WIN
                nc.sync.dma_start(dens_d[:, blk:blk + WIN], cur[0][:, 0:512])
                fin = finp.tile([128, WIN], F16, tag="fin",
                                name=f"fin{b}_{W}_{hl}")
                nc.vector.tensor_copy(fin[:], avs[hl][:])
                nc.sync.dma_start(
                    fins_d[hl * E:(hl + 1) * E,
                           b * S + W * WIN:b * S + (W + 1) * WIN], fin[:])

        def _emit_av(b, W, item, avs):
            hl, p, e2 = item
            for h2 in range(2):
                c = 2 * p + h2
                jlo = max(0, 128 * c - 512 * W)
                nc.tensor.matmul(
                    avs[hl][:, jlo:WIN],
                    vh[b][:, c * 256 + hl * 128:c * 256 + (hl + 1) * 128],
                    e2[:, h2 * 512 + jlo:(h2 + 1) * 512],
                    start=(c == 0), stop=(c == 4 * W + 3))

        # stage_b(0, w=0) first; all remaining stage-B windows are drip-fed
        # as filler into the stage_c emission stream (one quantum per
        # pair-step) so PE/ACT/DVE pipelines start early and stay full.
        for q in stage_b_quanta([(0, 0)]):
            q()
        rest = [(0, w) for w in range(1, NW)] + [(1, w) for w in range(NW)]
        filler = stage_b_quanta(rest)
        for W in range(NW):
            stage_c(0, W, filler=filler)
        for q in filler:
            q()
        for W in range(NW):
            stage_c(1, W)

    nc.compile()
    return nc


def _get_nc():
    if "nc" not in _CACHE:
        _CACHE["nc"] = _build()
    return _CACHE["nc"]


def _host_inputs(q, W_q, W_k, W_v, W_o):
    """Shared (core-independent) host-side prep."""
    qT = np.ascontiguousarray(q.reshape(B * S, E).T).astype(np.float16)

    half = D // 2
    inv = (1.0 / (10000.0 ** (np.arange(half, dtype=np.float64) * 2.0 / D)))
    ang = np.arange(S, dtype=np.float64)[None, :] * inv[:, None]   # [half, S]
    cosT = np.repeat(np.cos(ang), 2, axis=0)                       # [D, S]
    sinT = np.repeat(np.sin(ang), 2, axis=0)
    cs = np.empty((D, 2 * S), dtype=np.float16)
    for w in range(NW):
        cs[:, w * 1024:w * 1024 + 512] = cosT[:, w * 512:(w + 1) * 512]
        cs[:, w * 1024 + 512:(w + 1) * 1024] = sinT[:, w * 512:(w + 1) * 512]
    # tril[t, j] = 1 if j >= t  (keep q >= t within the diagonal block)
    tril = np.tril(np.ones((128, 128), dtype=np.float16)).T
    tril = np.ascontiguousarray(tril)
    return qT, cs, tril


def _swap_neg(w):
    """W' columns: w2[:, 2i] = -w[:, 2i+1], w2[:, 2i+1] = w[:, 2i]."""
    w2 = np.empty_like(w)
    w2[:, 0::2] = -w[:, 1::2]
    w2[:, 1::2] = w[:, 0::2]
    return w2


def kernel(q, W_q, W_k, W_v, W_o):
    q = np.asarray(q, dtype=np.float32)
    W_q = np.asarray(W_q, dtype=np.float64)
    W_k = np.asarray(W_k, dtype=np.float64)
    W_v = np.asarray(W_v, dtype=np.float64)
    W_o = np.asarray(W_o, dtype=np.float64)

    nc = _get_nc()
    qT, cs, tril = _host_inputs(q, W_q, W_k, W_v, W_o)

    in_maps = []
    for c in range(NCORES):
        wqk = np.empty((E, 8 * D), dtype=np.float16)
        wf2 = np.empty((E, HPC * E), dtype=np.float16)
        for hl in range(HPC):
            h = c * HPC + hl
            for kind, Wm in ((0, W_q), (1, W_k)):
                wslc = Wm[:, h * D:(h + 1) * D]
                ja = (hl * 2 + kind) * 256
                wqk[:, ja:ja + D] = wslc
                wqk[:, ja + D:ja + 2 * D] = _swap_neg(wslc)
            wf2[:, hl * E:(hl + 1) * E] = (
                W_v[:, h * D:(h + 1) * D] @ W_o[h * D:(h + 1) * D, :])
        in_maps.append({
            "qT": qT, "wqk": wqk, "wf2": wf2, "cs": cs, "trilT": tril,
        })

    res = bass_utils.run_bass_kernel_spmd(
        nc, in_maps, core_ids=list(range(NCORES)),
        trace=bool(int(os.environ.get("KERNEL_TRACE", "0"))))
    _CACHE["last_result"] = res

    out = np.zeros((B, S, E), dtype=np.float64)
    for r in res.results:
        out += _combine(r)
    return out.astype(np.float32)


def _combine(r):
    """Host-side normalization + head sum for one core's outputs."""
    fins = r["fins"].astype(np.float64).reshape(HPC, E, B, S)   # [hl,e,b,q]
    dens = r["dens"].astype(np.float64).reshape(
        128, B, NW, HPC, WIN)                                   # [t,b,W,hl,j]
    den = dens.sum(axis=0)                                      # [b,W,hl,j]
    den = den.transpose(0, 2, 1, 3).reshape(B, HPC, S)          # [b,hl,q]
    return np.einsum("lebq->bqe", fins / den.transpose(1, 0, 2)[:, None, :, :])
